# revision 17
# baseline (speedup 1.0000x reference)
"""Trainium2 Bass kernel for the hyperbolic Kuramoto-NCA message-passing net.

Sharding (8 NeuronCores, SPMD): each core owns a 512-column shard of every
[N,N] matrix, stored transposed: tile[u, v] = Mat[v, shard[u]] (4 partition
blocks of 128).  Symmetric quantities (gram, distance, alpha, phase outer
product) are identical in this layout; adjacency arrives per-core as
0.5*A[:, shard]^T.  The PE contracts over partitions = the j-sum of message
passing; partials combine with ReduceScatter/AllGather collectives.

All transcendentals use the single natural_log_exp ACT table set: sqrt(q) is
exp(0.5 ln q) (exact 0 at q=0), tanh is synthesized from Exp, the phase
rotation uses host-supplied cos/sin of theta0 advanced by polynomial
cos/sinc of the small increment, exp-map cosh/sinh via polynomials in n^2,
reciprocals via the DVE Newton-Raphson custom op.
"""

import numpy as np

import concourse.bass as bass
import concourse.mybir as mybir
import concourse.tile as tile
from concourse import bacc
from concourse.bass import ds, ts
from concourse.bass_utils import run_bass_kernel_spmd
from concourse.masks import make_identity

dt = mybir.dt
F = mybir.ActivationFunctionType
ALU = mybir.AluOpType

N = 4096
NC = 8
SH = N // NC
NB = SH // 128
NQ = 4
QW = N // NQ              # 1024
DP = 17
H = 64
NUM_STEPS = 2

DT_PH = 0.1
DT_NCA = 0.1
EPS = 1e-7
EPS_S = float(np.sqrt(EPS))
KAPPA_DT = DT_PH * 2.0 / N

F32 = dt.float32
F16 = dt.float16

CH_COEF = [1.0, 1.0 / 2, 1.0 / 24, 1.0 / 720, 1.0 / 40320]
SHC_COEF = [1.0, 1.0 / 6, 1.0 / 120, 1.0 / 5040, 1.0 / 362880]
CB_COEF = [1.0, -1.0 / 2, 1.0 / 24, -1.0 / 720]
SB_COEF = [1.0, -1.0 / 6, 1.0 / 120, -1.0 / 5040]


def build_nc():
    nc = bacc.Bacc("TRN2", target_bir_lowering=False, debug=False, num_devices=NC)
    # Pin every activation to the natural_log_exp set (Square/Ln/Exp/Copy all
    # live there) so insert_act_table_loads emits exactly one table load.
    import concourse.hw_specs as _hs
    _tabs = _hs.get_activation_tables(nc.m.arch)
    for _k in list(_tabs):
        if _k != "natural_log_exp_and_others":
            _tabs[_k] = set()

    at_in = nc.dram_tensor("at", [SH, N], F32, kind="ExternalInput")
    yt_in = nc.dram_tensor("yt", [DP, N], F32, kind="ExternalInput")
    xtsh_in = nc.dram_tensor("xtsh", [DP, SH], F32, kind="ExternalInput")
    xrows_in = nc.dram_tensor("xrows", [128, NB * DP], F16, kind="ExternalInput")
    yrows_in = nc.dram_tensor("yrows", [128, NB * DP], F16, kind="ExternalInput")
    sgnr_in = nc.dram_tensor("sgnr", [128, DP], F32, kind="ExternalInput")
    csv_in = nc.dram_tensor("csv", [128, 32], F32, kind="ExternalInput")
    snv_in = nc.dram_tensor("snv", [128, 32], F32, kind="ExternalInput")
    omv_in = nc.dram_tensor("omv", [128, 32], F32, kind="ExternalInput")
    cssh_in = nc.dram_tensor("cssh", [128, NB], F32, kind="ExternalInput")
    snsh_in = nc.dram_tensor("snsh", [128, NB], F32, kind="ExternalInput")
    omsh_in = nc.dram_tensor("omsh", [128, NB], F32, kind="ExternalInput")
    w1_in = nc.dram_tensor("w1", [DP, H], F32, kind="ExternalInput")
    b1_in = nc.dram_tensor("b1", [H, 1], F32, kind="ExternalInput")
    w2_in = nc.dram_tensor("w2", [H, DP], F32, kind="ExternalInput")
    konst_in = nc.dram_tensor("konst", [128, 3], F32, kind="ExternalInput")
    onesrow_in = nc.dram_tensor("onesrow", [1, N], F32, kind="ExternalInput")
    b2_in = nc.dram_tensor("b2", [DP, 1], F32, kind="ExternalInput")

    mt_out = nc.dram_tensor("mt", [SH, N], F32, kind="ExternalOutput")
    hts_out = nc.dram_tensor("hts", [DP, SH], F32, kind="ExternalOutput")
    r2_out = nc.dram_tensor("r2", [1, 1], F32, kind="ExternalOutput")

    rs1_in = nc.dram_tensor("rs1_in", [NC, 2, SH], F32)
    rs1_out = nc.dram_tensor("rs1_out", [2, SH], F32)
    ag1_out = nc.dram_tensor("ag1_out", [NC, 2, SH], F32, addr_space="Shared")
    rs2_in = nc.dram_tensor("rs2_in", [NC, 2 * DP, SH], F32)
    rs2_out = nc.dram_tensor("rs2_out", [2 * DP, SH], F32)
    ag2_in = nc.dram_tensor("ag2_in", [DP, SH], F32)
    ag2_out = nc.dram_tensor("ag2_out", [NC, DP, SH], F32, addr_space="Shared")

    RG = [list(range(NC))]

    with tile.TileContext(nc) as tc:
        with (
            tc.tile_pool(name="pers", bufs=1) as pers,
            tc.tile_pool(name="tmp", bufs=1) as tmp,
            tc.tile_pool(name="sm", bufs=1) as sm,
            tc.tile_pool(name="tl", bufs=3) as tl,
            tc.tile_pool(name="psG", bufs=4, space="PSUM") as psG,
            tc.tile_pool(name="psS", bufs=4, space="PSUM") as psS,
        ):
            # ---- consolidated small tensors (parks) ----
            pA = sm.tile([128, 1024], F32, tag="pA")    # consts / weights / vectors
            pU = sm.tile([128, 512], F32, tag="pU")     # phase-update scratch
            pB = sm.tile([128, 512], F16, tag="pB")     # f16 lhsT data
            YT = sm.tile([DP, N], F32, tag="YT")
            cssn1 = sm.tile([3, N], F32, tag="cssn1")
            lhsT3 = sm.tile([3, SH], F32, tag="lhsT3")
            xtsh = [sm.tile([DP, SH], F32, tag=f"xtsh{i}", name=f"xtsh{i}")
                    for i in range(2)]
            ytsh = sm.tile([DP, SH], F32, tag="ytsh")

            ident = pA[:, 0:128]
            w1 = pA[0:DP, 128:192]
            w2 = pA[0:H, 192:209]
            b1 = pA[0:H, 209:210]
            b2 = pA[0:DP, 210:211]
            sgn17 = pA[0:DP, 211:212]
            ones17 = pA[0:DP, 212:213]
            cm1 = pA[:, 213:214]
            ones128 = pA[:, 214:215]
            ones117 = pA[0:1, 215:232]
            csv = [pA[:, 232 + 32 * i:264 + 32 * i] for i in range(2)]
            snv = [pA[:, 296 + 32 * i:328 + 32 * i] for i in range(2)]
            omv = pA[:, 360:392]
            cssh = [pA[:, 392 + NB * i:396 + NB * i] for i in range(2)]
            snsh = [pA[:, 400 + NB * i:404 + NB * i] for i in range(2)]
            omsh = pA[:, 408:412]
            red2 = pA[:, 412:414]
            msq = pA[0:1, 489:491]
            r2t = pA[0:1, 416:417]
            rsnv = pA[:, 417:449]
            rcsv = pA[:, 449:481]
            rsns = pA[:, 481:485]
            rcss = pA[:, 485:489]

            xrows = [pB[:, 68 * i:68 * (i + 1)] for i in range(2)]
            cssh16 = [pB[:, 136 + NB * i:140 + NB * i] for i in range(2)]
            snsh16 = [pB[:, 144 + NB * i:148 + NB * i] for i in range(2)]
            yrows = [pB[:, 152 + 68 * i:220 + 68 * i] for i in range(2)]
            sncs2 = [pB[:, 288 + 8 * i:296 + 8 * i] for i in range(2)]
            sgnr = pA[:, 491:508]

            nc.sync.dma_start(YT[:], yt_in.ap())
            nc.sync.dma_start(xtsh[0][:], xtsh_in.ap())
            nc.sync.dma_start(xrows[0], xrows_in.ap())
            nc.sync.dma_start(yrows[0], yrows_in.ap())
            nc.sync.dma_start(sgnr[0:128, :], sgnr_in.ap())
            nc.sync.dma_start(csv[0], csv_in.ap())
            nc.sync.dma_start(snv[0], snv_in.ap())
            nc.sync.dma_start(omv, omv_in.ap())
            nc.sync.dma_start(cssh[0], cssh_in.ap())
            nc.sync.dma_start(snsh[0], snsh_in.ap())
            nc.sync.dma_start(omsh, omsh_in.ap())
            nc.sync.dma_start(w1, w1_in.ap())
            nc.sync.dma_start(b1, b1_in.ap())
            nc.sync.dma_start(w2, w2_in.ap())
            nc.sync.dma_start(b2, b2_in.ap())
            nc.sync.dma_start(sgn17, konst_in[0:DP, 0:1])
            nc.sync.dma_start(ones17, konst_in[0:DP, 1:2])
            nc.sync.dma_start(cm1, konst_in[:, 2:3])
            nc.sync.dma_start(ones128, konst_in[:, 1:2])
            nc.sync.dma_start(ones117, onesrow_in[0:1, 0:DP])
            nc.sync.dma_start(cssn1[2:3, :], onesrow_in.ap())
            nc.sync.dma_start(lhsT3[2:3, :], onesrow_in[0:1, 0:SH])
            make_identity(nc, ident)
            nc.vector.tensor_scalar(ytsh[:], xtsh[0][:], sgn17, None, ALU.mult)
            nc.vector.tensor_copy(cssh16[0], cssh[0])
            nc.vector.tensor_copy(snsh16[0], snsh[0])
            for b in range(NB):
                nc.vector.tensor_copy(sncs2[0][:, 2 * b:2 * b + 1], snsh[0][:, b:b + 1])
                nc.vector.tensor_copy(sncs2[0][:, 2 * b + 1:2 * b + 2], cssh[0][:, b:b + 1])

            # persistent per-step: C (becomes B in-place) and alpha, f16
            cb_t = [pers.tile([128, N], F16, tag=f"cb{b}", name=f"cb{b}")
                    for b in range(NB)]
            al_t = [pers.tile([128, N], F16, tag=f"al{b}", name=f"al{b}")
                    for b in range(NB)]

            def horner(out_ap, y_ap, coef, P, W, scr_a=None, scr_b=None):
                if scr_a is None:
                    a_t = tl.tile([P, W], F32, tag="tl", name="ha")
                    b_t_ = tl.tile([P, W], F32, tag="tl", name="hb")
                    scr_a, scr_b = a_t[:], b_t_[:]
                nc.vector.tensor_scalar(scr_a, y_ap, float(coef[-1]), float(coef[-2]),
                                        ALU.mult, ALU.add)
                for k in range(len(coef) - 3, -1, -1):
                    nc.vector.tensor_tensor(scr_b, scr_a, y_ap, ALU.mult)
                    nc.vector.tensor_scalar(out_ap if k == 0 else scr_a,
                                            scr_b, float(coef[k]), None, ALU.add)

            for step in range(NUM_STEPS):
                last = step == NUM_STEPS - 1
                cs_c, sn_c = csv[step], snv[step]
                cssh16_c, snsh16_c = cssh16[step], snsh16[step]
                xtsh_c = xtsh[step]
                xrows_c = xrows[step]

                # ============ phase A: G -> z -> chain -> C', alpha ============
                for b in range(NB):
                    lhsG = xtsh_c[:, ds(b * 128, 128)]
                    for q in range(NQ):
                        sl = ds(q * QW, QW)
                        z32 = tmp.tile([128, QW], F32, tag="z32", name="z32", bufs=2)
                        for m in range(2):
                            gp = psG.tile([128, 512], F32, tag="gq", name="gq")
                            nc.tensor.matmul(gp[:], lhsG,
                                             YT[:, ds(q * QW + m * 512, 512)],
                                             start=True, stop=True)
                            nc.vector.tensor_scalar(z32[:, ts(m, 512)], gp[:], -1.0,
                                                    1.0, ALU.mult, ALU.max)
                        sq = tmp.tile([128, QW], F32, tag="sq", name="sq")
                        nc.scalar.activation(sq[:], z32[:], F.Square)
                        lnq = tmp.tile([128, QW], F32, tag="lnq", name="lnq")
                        nc.scalar.activation(lnq[:], sq[:], F.Ln, bias=cm1)
                        s0 = tmp.tile([128, QW], F32, tag="s0", name="s0", bufs=2)
                        nc.scalar.activation(s0[:], lnq[:], F.Exp, scale=0.5)
                        wm = tmp.tile([128, QW], F32, tag="wm", name="wm", bufs=2)
                        nc.gpsimd.tensor_tensor(wm[:], z32[:], s0[:], ALU.subtract)
                        dneg = tmp.tile([128, QW], F32, tag="dneg", name="dneg", bufs=2)
                        nc.scalar.activation(dneg[:], wm[:], F.Ln)
                        S = tmp.tile([128, QW], F32, tag="S", name="S", bufs=2)
                        nc.scalar.activation(S[:], dneg[:], F.Exp, scale=2.0)
                        at_q = tmp.tile([128, QW], F32, tag="at", name="at", bufs=3)
                        nc.sync.dma_start(at_q[:], at_in[ds(b * 128, 128), sl])
                        nc.gpsimd.tensor_tensor(cb_t[b][:, sl], at_q[:], S[:], ALU.mult)
                        s1m = tmp.tile([128, QW], F32, tag="s1m", name="s1m")
                        nc.vector.tensor_scalar(s1m[:], s0[:], EPS_S, None, ALU.max)
                        rec = tmp.tile([128, QW], F32, tag="rec", name="rec")
                        nc.vector.reciprocal_approx_fast(rec[:], s1m[:])
                        nc.vector.scalar_tensor_tensor(al_t[b][:, sl], dneg[:], -1.0,
                                                       rec[:], ALU.mult, ALU.mult)

                # ============ Kuramoto matvec partials + RS1 + AG1 ============
                sncs2_c = sncs2[step]
                for v in range(NC):
                    r_ps = psS.tile([2, 512], F32, tag="ps", name="r_ps")
                    for b in range(NB):
                        nc.tensor.matmul(r_ps[:], sncs2_c[:, 2 * b:2 * b + 2],
                                         cb_t[b][:, ts(v, 512)],
                                         start=(b == 0), stop=(b == NB - 1))
                    rsb = tl.tile([2, 512], F32, tag="tl", name="rsb")
                    nc.scalar.copy(rsb[:], r_ps[:])
                    nc.sync.dma_start(rs1_in[v, :, :], rsb[:])
                nc.gpsimd.collective_compute(
                    "ReduceScatter", ALU.add, replica_groups=RG,
                    ins=[rs1_in.ap().opt()], outs=[rs1_out.ap().opt()])
                nc.gpsimd.collective_compute(
                    "AllGather", ALU.bypass, replica_groups=RG,
                    ins=[rs1_out.ap().opt()], outs=[ag1_out.ap().opt()])

                # ---------- phase updates ----------
                for r in range(NC):
                    nc.sync.dma_start(rsnv[ds(16 * r, 16), :],
                                      ag1_out[r, 0, :].rearrange("(a b) -> a b", a=16))
                    nc.sync.dma_start(rcsv[ds(16 * r, 16), :],
                                      ag1_out[r, 1, :].rearrange("(a b) -> a b", a=16))
                for b in range(NB):
                    nc.sync.dma_start(rsns[:, b:b + 1],
                                      rs1_out[0, ds(b * 128, 128)]
                                      .rearrange("(a b) -> a b", b=1))
                    nc.sync.dma_start(rcss[:, b:b + 1],
                                      rs1_out[1, ds(b * 128, 128)]
                                      .rearrange("(a b) -> a b", b=1))

                def phase_update(cs0, sn0, om, rsn, rcs, cs1, sn1, w, base):
                    t1 = pU[:, base:base + w]
                    t2 = pU[:, base + w:base + 2 * w]
                    bb = pU[:, base + 2 * w:base + 3 * w]
                    y = pU[:, base + 3 * w:base + 4 * w]
                    cb_ = pU[:, base + 4 * w:base + 5 * w]
                    sbr = pU[:, base + 5 * w:base + 6 * w]
                    sb_ = pU[:, base + 6 * w:base + 7 * w]
                    e1 = pU[:, base + 7 * w:base + 8 * w]
                    e2 = pU[:, base + 8 * w:base + 9 * w]
                    ha = pU[:, base + 9 * w:base + 10 * w]
                    hb = pU[:, base + 10 * w:base + 11 * w]
                    nc.vector.tensor_tensor(t1, cs0, rsn, ALU.mult)
                    nc.vector.tensor_tensor(t2, sn0, rcs, ALU.mult)
                    nc.vector.tensor_tensor(t1, t1, t2, ALU.subtract)
                    nc.vector.scalar_tensor_tensor(bb, t1, KAPPA_DT, om,
                                                   ALU.mult, ALU.add)
                    nc.vector.tensor_tensor(y, bb, bb, ALU.mult)
                    horner(cb_, y, CB_COEF, 128, w, ha, hb)
                    horner(sbr, y, SB_COEF, 128, w, ha, hb)
                    nc.vector.tensor_tensor(sb_, sbr, bb, ALU.mult)
                    nc.vector.tensor_tensor(e1, cs0, cb_, ALU.mult)
                    nc.vector.tensor_tensor(e2, sn0, sb_, ALU.mult)
                    nc.vector.tensor_tensor(cs1, e1, e2, ALU.subtract)
                    nc.vector.tensor_tensor(e1, sn0, cb_, ALU.mult)
                    nc.vector.tensor_tensor(e2, cs0, sb_, ALU.mult)
                    nc.vector.tensor_tensor(sn1, e1, e2, ALU.add)

                phase_update(cs_c, sn_c, omv, rsnv, rcsv,
                             csv[1 - step], snv[1 - step], 32, 0)
                phase_update(cssh[step], snsh[step], omsh, rsns, rcss,
                             cssh[1 - step], snsh[1 - step], NB, 352)
                cs2v, sn2v = csv[1 - step], snv[1 - step]
                cs2s, sn2s = cssh[1 - step], snsh[1 - step]
                if not last:
                    nc.vector.tensor_copy(cssh16[1 - step], cs2s)
                    nc.vector.tensor_copy(snsh16[1 - step], sn2s)
                    for b in range(NB):
                        nc.vector.tensor_copy(sncs2[1 - step][:, 2 * b:2 * b + 1],
                                              sn2s[:, b:b + 1])
                        nc.vector.tensor_copy(sncs2[1 - step][:, 2 * b + 1:2 * b + 2],
                                              cs2s[:, b:b + 1])

                for src, row in ((cs2v, 0), (sn2v, 1)):
                    # v128 is p-major (node = 32p + f): partition-major DMA stream
                    # of [128, 32] is already node order
                    nc.sync.dma_start(cssn1[row:row + 1, :], src)
                for src, row in ((cs2s, 0), (sn2s, 1)):
                    tp = psS.tile([4, 128], F32, tag="ps", name="tps")
                    nc.tensor.transpose(tp[:], src, ident)
                    tpsb = tl.tile([4, 128], F32, tag="tl", name="tpsb2")
                    nc.scalar.copy(tpsb[:], tp[:])
                    nc.sync.dma_start(lhsT3[row:row + 1, :], tpsb[:])

                if last:
                    nc.vector.tensor_reduce(red2[:, 0:1], cs2v, mybir.AxisListType.X,
                                            ALU.add)
                    nc.vector.tensor_reduce(red2[:, 1:2], sn2v, mybir.AxisListType.X,
                                            ALU.add)
                    mm = psS.tile([1, 2], F32, tag="ps", name="mm")
                    nc.tensor.matmul(mm[:], ones128, red2, start=True, stop=True)
                    mmsb = pA[0:1, 414:416]
                    nc.scalar.copy(mmsb, mm[:])
                    nc.vector.tensor_tensor(msq, mmsb, mmsb, ALU.mult)
                    nc.vector.tensor_reduce(r2t, msq, mybir.AxisListType.X, ALU.add)
                    nc.vector.tensor_scalar(r2t, r2t, 1.0 / (float(N) * N), None,
                                            ALU.mult)
                    nc.sync.dma_start(r2_out.ap(), r2t)

                # ============ phase B: O' -> M -> B (in-place over C) ============
                for b in range(NB):
                    for q in range(NQ):
                        sl = ds(q * QW, QW)
                        mh = tmp.tile([128, QW], F32, tag="mh", name="mh", bufs=2)
                        for m in range(2):
                            op = psG.tile([128, 512], F32, tag="gq", name="opq")
                            nc.tensor.matmul(op[:],
                                             lhsT3[:, ds(b * 128, 128)],
                                             cssn1[:, ds(q * QW + m * 512, 512)],
                                             start=True, stop=True)
                            nc.vector.tensor_tensor(
                                mh[:, ts(m, 512)],
                                cb_t[b][:, ds(q * QW + m * 512, 512)], op[:], ALU.mult)
                        if last:
                            nc.sync.dma_start(mt_out[ds(b * 128, 128), sl], mh[:])
                        # B overwrites C in place (chunkwise WAR handled by Tile)
                        nc.gpsimd.tensor_tensor(cb_t[b][:, sl], mh[:], al_t[b][:, sl],
                                                ALU.mult)

                # ============ mT + rowe partials -> RS2 ============
                yrows_c = yrows[step]
                for v in range(NC):
                    macc = psS.tile([DP, 512], F32, tag="ps", name="macc")
                    pacc = psS.tile([DP, 512], F32, tag="ps", name="pacc")
                    for b in range(NB):
                        nc.tensor.matmul(macc[:], xrows_c[:, ds(b * DP, DP)],
                                         cb_t[b][:, ts(v, 512)],
                                         start=(b == 0), stop=(b == NB - 1))
                    for b in range(NB):
                        nc.tensor.matmul(pacc[:], yrows_c[:, ds(b * DP, DP)],
                                         cb_t[b][:, ts(v, 512)],
                                         start=(b == 0), stop=(b == NB - 1))
                    msb = tl.tile([DP, 512], F32, tag="tl", name="msb")
                    nc.scalar.copy(msb[:], macc[:])
                    psb = tl.tile([DP, 512], F32, tag="tl", name="psb")
                    nc.scalar.copy(psb[:], pacc[:])
                    nc.sync.dma_start(rs2_in[v, 0:DP, :], msb[:])
                    nc.sync.dma_start(rs2_in[v, DP:2 * DP, :], psb[:])
                nc.gpsimd.collective_compute(
                    "ReduceScatter", ALU.add, replica_groups=RG,
                    ins=[rs2_in.ap().opt()], outs=[rs2_out.ap().opt()])

                # ============ phase C: shard NCA tail ============
                mshm = tl.tile([DP, SH], F32, tag="tlm", name="mshm", bufs=1)
                nc.sync.dma_start(mshm[:], rs2_out[0:DP, :])
                mshp = tl.tile([DP, SH], F32, tag="tl", name="mshp")
                nc.sync.dma_start(mshp[:], rs2_out[DP:2 * DP, :])
                prode = tl.tile([DP, SH], F32, tag="tl", name="prode")
                nc.vector.tensor_tensor(prode[:], mshp[:], xtsh_c[:], ALU.mult)
                re_ps = psS.tile([1, SH], F32, tag="ps", name="re_ps")
                nc.tensor.matmul(re_ps[:], ones17, prode[:], start=True, stop=True)
                re_sb = tl.tile([1, SH], F32, tag="tl", name="re_sb")
                nc.vector.tensor_copy(re_sb[:], re_ps[:])
                rb = psS.tile([DP, SH], F32, tag="ps", name="rb")
                nc.tensor.matmul(rb[:], ones117, re_sb[:], start=True, stop=True)
                mx = tl.tile([DP, SH], F32, tag="tl", name="mx")
                nc.vector.tensor_tensor(mx[:], rb[:], xtsh_c[:], ALU.mult)
                mfin = tl.tile([DP, SH], F32, tag="tlm2", name="mfin", bufs=1)
                nc.vector.tensor_tensor(mfin[:], mshm[:], mx[:], ALU.add)

                t1p = psS.tile([H, SH], F32, tag="ps", name="t1p")
                nc.tensor.matmul(t1p[:], w1, mfin[:], start=True, stop=True)
                tc1 = tl.tile([H, SH], F32, tag="tl", name="tc1")
                nc.vector.tensor_scalar(tc1[:], t1p[:], b1, 30.0, ALU.add, ALU.min)
                nc.vector.tensor_scalar(tc1[:], tc1[:], -30.0, None, ALU.max)
                te = tl.tile([H, SH], F32, tag="tl", name="te")
                nc.scalar.activation(te[:], tc1[:], F.Exp, scale=2.0)
                nc.vector.tensor_scalar(te[:], te[:], 1.0, None, ALU.add)
                trc = tl.tile([H, SH], F32, tag="tl", name="trc")
                nc.vector.reciprocal_approx_fast(trc[:], te[:])
                tht = tl.tile([H, SH], F32, tag="tl", name="tht")
                nc.vector.tensor_scalar(tht[:], trc[:], -2.0, 1.0, ALU.mult, ALU.add)

                vp = psS.tile([DP, SH], F32, tag="ps", name="vp")
                nc.tensor.matmul(vp[:], w2, tht[:], start=True, stop=True)
                vb = tl.tile([DP, SH], F32, tag="tlvb", name="vb", bufs=1)
                nc.vector.tensor_scalar(vb[:], vp[:], b2, None, ALU.add)

                prod = tl.tile([DP, SH], F32, tag="tl", name="prod")
                nc.vector.tensor_tensor(prod[:], ytsh[:], vb[:], ALU.mult)
                mdp = psS.tile([1, SH], F32, tag="ps", name="mdp")
                nc.tensor.matmul(mdp[:], ones17, prod[:], start=True, stop=True)
                md = tl.tile([1, SH], F32, tag="tl", name="md")
                nc.vector.tensor_copy(md[:], mdp[:])
                mdb = psS.tile([DP, SH], F32, tag="ps", name="mdb")
                nc.tensor.matmul(mdb[:], ones117, md[:], start=True, stop=True)
                pmx = tl.tile([DP, SH], F32, tag="tl", name="pmx")
                nc.vector.tensor_tensor(pmx[:], mdb[:], xtsh_c[:], ALU.mult)
                vpj = tl.tile([DP, SH], F32, tag="tlvpj", name="vpj", bufs=1)
                nc.vector.tensor_tensor(vpj[:], vb[:], pmx[:], ALU.add)

                prod2 = tl.tile([DP, SH], F32, tag="tl", name="prod2")
                nc.vector.scalar_tensor_tensor(prod2[:], vpj[:], sgn17, vpj[:],
                                               ALU.mult, ALU.mult)
                nnp = psS.tile([1, SH], F32, tag="ps", name="nnp")
                nc.tensor.matmul(nnp[:], ones17, prod2[:], start=True, stop=True)
                y1 = tl.tile([1, SH], F32, tag="tly", name="y1", bufs=1)
                nc.vector.tensor_scalar(y1[:], nnp[:], DT_NCA * DT_NCA, EPS,
                                        ALU.mult, ALU.max)
                chp = tl.tile([1, SH], F32, tag="tlch", name="chp", bufs=1)
                horner(chp[:], y1[:], CH_COEF, 1, SH)
                shp = tl.tile([1, SH], F32, tag="tlsh", name="shp", bufs=1)
                horner(shp[:], y1[:], SHC_COEF, 1, SH)

                chb = psS.tile([DP, SH], F32, tag="ps", name="chb")
                nc.tensor.matmul(chb[:], ones117, chp[:], start=True, stop=True)
                shb = psS.tile([DP, SH], F32, tag="ps", name="shb")
                nc.tensor.matmul(shb[:], ones117, shp[:], start=True, stop=True)
                hA = tl.tile([DP, SH], F32, tag="tl", name="hA")
                nc.vector.tensor_tensor(hA[:], xtsh_c[:], chb[:], ALU.mult)
                hB = tl.tile([DP, SH], F32, tag="tl", name="hB")
                nc.vector.scalar_tensor_tensor(hB[:], vpj[:], DT_NCA, shb[:],
                                               ALU.mult, ALU.mult)
                h_new = xtsh[1 - step]
                nc.vector.tensor_tensor(h_new[:], hA[:], hB[:], ALU.add)

                if not last:
                    nc.vector.tensor_scalar(ytsh[:], h_new[:], sgn17, None, ALU.mult)
                    for b in range(NB):
                        xt_ps = psS.tile([128, DP], F32, tag="ps", name="xt_ps")
                        nc.tensor.transpose(xt_ps[:], h_new[:, ds(b * 128, 128)],
                                            ident[0:DP, 0:DP])
                        nc.vector.tensor_copy(xrows[1 - step][:, ds(b * DP, DP)],
                                              xt_ps[:])
                        nc.vector.tensor_tensor(yrows[1 - step][:, ds(b * DP, DP)],
                                                xt_ps[:], sgnr, ALU.mult)
                    nc.sync.dma_start(ag2_in.ap(), h_new[:])
                    nc.gpsimd.collective_compute(
                        "AllGather", ALU.bypass, replica_groups=RG,
                        ins=[ag2_in.ap().opt()], outs=[ag2_out.ap().opt()])
                    for r in range(NC):
                        nc.sync.dma_start(YT[:, ds(r * SH, SH)], ag2_out[r, :, :])
                    nc.vector.tensor_scalar(YT[0:1, :], YT[0:1, :], -1.0, None,
                                            ALU.mult)
                else:
                    nc.sync.dma_start(hts_out.ap(), h_new[:])

    nc.finalize()
    return nc


_CACHED_NC = None
LAST_RESULT = None


def _get_nc():
    global _CACHED_NC
    if _CACHED_NC is None:
        _CACHED_NC = build_nc()
    return _CACHED_NC


def kernel(adjacency, h_states, theta, omega, W1, b1, W2, b2, num_steps):
    adjacency = np.asarray(adjacency, dtype=np.float32)
    h0 = np.asarray(h_states, dtype=np.float32)
    theta = np.asarray(theta, dtype=np.float32)
    omega = np.asarray(omega, dtype=np.float32)
    W1 = np.ascontiguousarray(np.asarray(W1, dtype=np.float32))
    b1 = np.asarray(b1, dtype=np.float32)
    W2 = np.ascontiguousarray(np.asarray(W2, dtype=np.float32))
    b2 = np.asarray(b2, dtype=np.float32)
    assert int(num_steps) == NUM_STEPS

    xt = np.ascontiguousarray(h0.T)
    sgn = np.ones((DP, 1), np.float32)
    sgn[0, 0] = -1.0
    yt = np.ascontiguousarray(xt * sgn)
    cs0 = np.cos(theta).astype(np.float32)
    sn0 = np.sin(theta).astype(np.float32)

    konst = np.ones((128, 3), np.float32)
    konst[0, 0] = -1.0
    konst[:, 2] = -1.0
    common = {
        "yt": yt, "konst": konst, "onesrow": np.ones((1, N), np.float32),
        "sgnr": np.tile(sgn.T, (128, 1)).astype(np.float32),
        "csv": cs0.reshape(128, 32).copy(),
        "snv": sn0.reshape(128, 32).copy(),
        "omv": (DT_PH * omega).astype(np.float32).reshape(128, 32).copy(),
        "w1": W1, "b1": b1.reshape(H, 1).astype(np.float32),
        "w2": W2, "b2": b2.reshape(DP, 1).astype(np.float32),
    }
    in_maps = []
    for c in range(NC):
        sl = slice(c * SH, (c + 1) * SH)
        in_maps.append(dict(
            common,
            at=np.ascontiguousarray(0.5 * adjacency[:, sl].T),
            xtsh=np.ascontiguousarray(h0[sl].T),
            xrows=np.ascontiguousarray(
                h0[sl].reshape(NB, 128, DP).transpose(1, 0, 2).reshape(128, NB * DP)
            ).astype(np.float16),
            yrows=np.ascontiguousarray(
                (h0[sl] * sgn.T).reshape(NB, 128, DP).transpose(1, 0, 2)
                .reshape(128, NB * DP)).astype(np.float16),
            cssh=np.ascontiguousarray(cs0[sl].reshape(NB, 128).T),
            snsh=np.ascontiguousarray(sn0[sl].reshape(NB, 128).T),
            omsh=np.ascontiguousarray((DT_PH * omega[sl]).reshape(NB, 128).T),
        ))

    import os
    kwargs = {}
    if os.environ.get("KERNEL_TRACE"):
        kwargs = dict(trace=True, trace_cores=[0])
    res = run_bass_kernel_spmd(_get_nc(), in_maps, core_ids=list(range(NC)), **kwargs)
    global LAST_RESULT
    LAST_RESULT = res

    M = np.concatenate([res.results[c]["mt"].T for c in range(NC)], axis=1)
    h = np.concatenate([res.results[c]["hts"].T for c in range(NC)], axis=0)
    r = np.sqrt(max(float(res.results[0]["r2"][0, 0]), 0.0))
    return h, np.ascontiguousarray(M), np.float32(r)
